# revision 21
# baseline (speedup 1.0000x reference)
"""DeepseekV3 decoder layer on 8 Trainium2 NeuronCores (Bass/Tile).

Sharding: sequence-parallel low-rank projections (local RMS, one AllGather),
tensor-parallel heads for q_b/kv_b/attention (2 heads/core, transposed-score
layout avoids attention transposes), AllGather of head outputs,
output-feature-sharded o_proj + residual, tiny AllReduce for post-LN stats,
AllGather of the normed MLP input, FF-sharded MLP with chunked ReduceScatter
overlapped with the down-projection.

Attention path matmuls run in float32r (full PE rate at N>=256, ~1.5e-4 rel
err); the MLP runs in bf16 (halves SBUF/DMA; error contribution ~1e-3).
RMS/ln weights and the rope de-interleave are folded into the weight
matrices host-side, so device RMS is exact and rope is 5 vector ops.
"""

import numpy as np

B, S, H = 1, 2048, 2048
NH, NOPE, ROPE, VHD = 16, 128, 64, 128
QHD = NOPE + ROPE
QLR, KVLR, FF = 1536, 512, 8192
SCALE = QHD ** -0.5
EPS = 1e-6
NC = 8
SS = S // NC            # 256: sequence / output-feature shard
FFS = FF // NC          # 1024: FF shard
P = 128
BS = QLR + KVLR + ROPE  # 2112: ag1 block rows per rank

TRACE = False           # test.py sets kernel.TRACE = True for profiling
DEBUG = False           # adds debug ExternalOutputs of stage boundaries

_CACHE = {}


def _tile_w(w):
    """[K, M] -> [K/128, ceil(M/128), 128, 128] contiguous blocks (zero-pad M)."""
    K, M = w.shape
    mc = -(-M // P)
    out = np.zeros((K // P, mc, P, P), np.float32)
    wp = np.zeros((K, mc * P), np.float32)
    wp[:, :M] = w
    for kt in range(K // P):
        for m in range(mc):
            out[kt, m] = wp[kt * P:(kt + 1) * P, m * P:(m + 1) * P]
    return out


def _build():
    if "nc" in _CACHE:
        return _CACHE["nc"]
    import concourse.mybir as mybir
    import concourse.tile as tile
    from concourse import bacc
    from concourse.masks import make_identity

    F32 = mybir.dt.float32
    F32R = mybir.dt.float32r
    BF16 = mybir.dt.bfloat16
    AF = mybir.ActivationFunctionType

    nc = bacc.Bacc("TRN2", target_bir_lowering=False, debug=False, num_devices=NC)

    def inp(name, shape, dt=F32):
        return nc.dram_tensor(name, list(shape), dt, kind="ExternalInput").ap()

    hT_s = inp("hT_s", [H, SS])
    hT_r = inp("hT_r", [SS, S])
    wq_a_t = inp("wq_a_t", [16, 12, P, P], F32R)
    wkv_a_t = inp("wkv_a_t", [16, 5, P, P], F32R)
    wq_b_t = inp("wq_b_t", [12, 3, P, P], F32R)
    wkv_b_t = inp("wkv_b_t", [4, 4, P, P], F32R)
    wo_t = inp("wo_t", [16, 2, P, P], F32R)
    wg_t = inp("wg_t", [16, 8, P, P], BF16)
    wu_t = inp("wu_t", [16, 8, P, P], BF16)
    wd_t = inp("wd_t", [8, 16, P, P], BF16)
    cossin = inp("cossin", [2 * P, S])        # rows 0:128 [cosT;cosT], 128:256 [sinT;sinT]
    cs_sh = inp("cs_sh", [P, SS])             # rows 0:64 cosT, 64:128 sinT (own shard)
    dmask = inp("dmask", [P, 4, 512])
    outT = nc.dram_tensor("outT", [SS, S], F32, kind="ExternalOutput").ap()

    RG = [list(range(NC))]
    dbg = {}
    if DEBUG:
        for nm, shp in [("dbg_ag1", [NC * BS, SS]), ("dbg_ag2", [NH * VHD, S]),
                        ("dbg_ag3", [H, S]), ("dbg_h2", [SS, S])]:
            dt = mybir.dt.bfloat16 if nm == "dbg_ag3" else F32
            dbg[nm] = nc.dram_tensor(nm, shp, dt, kind="ExternalOutput").ap()

    def r32(ap):
        return ap.bitcast(F32R)

    from contextlib import ExitStack
    with tile.TileContext(nc) as tc, ExitStack() as _stack:
        cpool = _stack.enter_context(tc.tile_pool(name="const", bufs=1))
        dpool = _stack.enter_context(tc.tile_pool(name="dram", bufs=1, space="DRAM"))
        ag1_in = dpool.tile([BS, SS], F32)
        ag1_out = dpool.tile([NC * BS, SS], F32, addr_space="Shared")
        ag2_in = dpool.tile([2 * VHD, S], F32)
        ag2_out = dpool.tile([NH * VHD, S], F32, addr_space="Shared")
        ar4_in = dpool.tile([1, S], F32)
        ar4_out = dpool.tile([1, S], F32, addr_space="Shared")
        ag3_in = dpool.tile([SS, S], BF16)
        ag3_out = dpool.tile([H, S], BF16, addr_space="Shared")
        rs_in = [dpool.tile([H, 512], BF16, name=f"rs_in{j}") for j in range(4)]
        rs_out = [dpool.tile([SS, 512], BF16, name=f"rs_out{j}") for j in range(4)]
        ones_f = cpool.tile([P, 1], F32)
        nc.vector.memset(ones_f[:], 1.0)
        ones_r = cpool.tile([P, 1], F32R)
        nc.vector.tensor_copy(ones_r[:], ones_f[:])
        ident_f = cpool.tile([P, P], F32)
        make_identity(nc, ident_f)
        ident_r = cpool.tile([P, P], F32R)
        nc.vector.tensor_copy(ident_r[:], ident_f[:])
        eps_t = cpool.tile([P, 1], F32)
        nc.vector.memset(eps_t[:], EPS)
        ones_k1f = cpool.tile([1, P], F32)
        nc.vector.memset(ones_k1f[:], 1.0)
        ones_k1 = cpool.tile([1, P], F32R)
        nc.vector.tensor_copy(ones_k1[:], ones_k1f[:])

        # ================= Stage A: seq-shard low-rank path =================
        with tc.tile_pool(name="sa", bufs=1) as sa, \
             tc.tile_pool(name="saw", bufs=4) as saw, \
             tc.tile_pool(name="pa", bufs=2, space="PSUM") as pa:
            with nc.named_scope("stageA"):
                xs = sa.tile([P, 16, SS], F32)
                nc.sync.dma_start(xs[:], hT_s.rearrange("(kt p) s -> p kt s", p=P))
                sq = sa.tile([P, 16, SS], F32R)
                nc.vector.tensor_mul(sq[:], xs[:], xs[:])
                msq_ps = pa.tile([1, SS], F32, tag="msq")
                for kt in range(16):
                    nc.tensor.matmul(msq_ps[:], ones_r[:], sq[:, kt],
                                     start=(kt == 0), stop=(kt == 15))
                r1s = sa.tile([1, SS], F32)
                nc.scalar.activation(r1s[:], msq_ps[:], AF.Sqrt, scale=1.0 / H, bias=eps_t[:1])
                r1 = sa.tile([1, SS], F32R)
                with nc.allow_low_precision(reason="f32r rounding of rms scale"):
                    nc.vector.reciprocal(r1[:], r1s[:])
                r1bp = pa.tile([P, SS], F32, tag="rb", name="r1bp")
                nc.tensor.matmul(r1bp[:], ones_k1[:], r1[:], start=True, stop=True)
                r1b = sa.tile([P, SS], F32)
                nc.vector.tensor_copy(r1b[:], r1bp[:])
                xn = sa.tile([P, 16, SS], F32R)
                nc.vector.tensor_mul(xn[:], xs[:],
                                     r1b[:, None, :].to_broadcast([P, 16, SS]))

                us = sa.tile([P, 12, SS], F32)
                for mc in range(12):
                    wt = saw.tile([P, 16, P], F32R, tag="aw")
                    nc.sync.dma_start(wt[:], wq_a_t[:, mc].rearrange("a p m -> p a m"))
                    ps = pa.tile([P, SS], F32, tag="amm")
                    for kt in range(16):
                        nc.tensor.matmul(ps[:], wt[:, kt], xn[:, kt],
                                         start=(kt == 0), stop=(kt == 15))
                    nc.vector.tensor_copy(us[:, mc], ps[:])
                cvs = sa.tile([P, 5, SS], F32)
                for mc in range(5):
                    wt = saw.tile([P, 16, P], F32R, tag="aw")
                    nc.sync.dma_start(wt[:], wkv_a_t[:, mc].rearrange("a p m -> p a m"))
                    ps = pa.tile([P, SS], F32, tag="amm")
                    for kt in range(16):
                        nc.tensor.matmul(ps[:], wt[:, kt], xn[:, kt],
                                         start=(kt == 0), stop=(kt == 15))
                    nc.vector.tensor_copy(cvs[:, mc], ps[:])

                sq2 = sa.tile([P, 12, SS], F32R)
                nc.vector.tensor_mul(sq2[:], us[:], us[:])
                msq2 = pa.tile([1, SS], F32, tag="msq")
                for mc in range(12):
                    nc.tensor.matmul(msq2[:], ones_r[:], sq2[:, mc],
                                     start=(mc == 0), stop=(mc == 11))
                r2s = sa.tile([1, SS], F32)
                nc.scalar.activation(r2s[:], msq2[:], AF.Sqrt, scale=1.0 / QLR, bias=eps_t[:1])
                r2 = sa.tile([1, SS], F32R)
                with nc.allow_low_precision(reason="f32r rounding of rms scale"):
                    nc.vector.reciprocal(r2[:], r2s[:])
                r2bp = pa.tile([P, SS], F32, tag="rb", name="r2bp")
                nc.tensor.matmul(r2bp[:], ones_k1[:], r2[:], start=True, stop=True)
                r2b = sa.tile([P, SS], F32)
                nc.vector.tensor_copy(r2b[:], r2bp[:])
                un = sa.tile([P, 12, SS], F32R)
                nc.vector.tensor_mul(un[:], us[:],
                                     r2b[:, None, :].to_broadcast([P, 12, SS]))

                sq3 = sa.tile([P, 4, SS], F32R)
                nc.vector.tensor_mul(sq3[:], cvs[:, :4], cvs[:, :4])
                msq3 = pa.tile([1, SS], F32, tag="msq")
                for mc in range(4):
                    nc.tensor.matmul(msq3[:], ones_r[:], sq3[:, mc],
                                     start=(mc == 0), stop=(mc == 3))
                r3s = sa.tile([1, SS], F32)
                nc.scalar.activation(r3s[:], msq3[:], AF.Sqrt, scale=1.0 / KVLR, bias=eps_t[:1])
                r3 = sa.tile([1, SS], F32R)
                with nc.allow_low_precision(reason="f32r rounding of rms scale"):
                    nc.vector.reciprocal(r3[:], r3s[:])
                r3bp = pa.tile([P, SS], F32, tag="rb", name="r3bp")
                nc.tensor.matmul(r3bp[:], ones_k1[:], r3[:], start=True, stop=True)
                r3b = sa.tile([P, SS], F32)
                nc.vector.tensor_copy(r3b[:], r3bp[:])
                ckn = sa.tile([P, 4, SS], F32R)
                nc.vector.tensor_mul(ckn[:], cvs[:, :4],
                                     r3b[:, None, :].to_broadcast([P, 4, SS]))

                # k_pe rope on cvs[:64, 4] (cs_sh rows 0:64 cos, 64:128 sin)
                cos_sh = sa.tile([64, SS], F32)
                nc.sync.dma_start(cos_sh[:], cs_sh[0:64, :])
                sin_sh = sa.tile([64, SS], F32)
                nc.sync.dma_start(sin_sh[:], cs_sh[64:128, :])
                ksw = sa.tile([64, SS], F32)
                nc.sync.dma_start(ksw[0:32, :], cvs[32:64, 4])
                nc.sync.dma_start(ksw[32:64, :], cvs[0:32, 4])
                kpe_n = sa.tile([64, SS], F32R)
                nc.vector.tensor_mul(kpe_n[:], cvs[:64, 4], cos_sh[:])
                t1 = sa.tile([64, SS], F32)
                nc.vector.tensor_mul(t1[:], ksw[:], sin_sh[:])
                nc.vector.tensor_add(kpe_n[:], kpe_n[:], t1[:])

                nc.sync.dma_start(
                    r32(ag1_in[0:QLR].rearrange("(mc p) s -> p mc s", p=P)),
                    un[:])
                nc.sync.dma_start(
                    r32(ag1_in[QLR:QLR + KVLR].rearrange(
                        "(mc p) s -> p mc s", p=P)), ckn[:])
                nc.sync.dma_start(r32(ag1_in[QLR + KVLR:BS]), kpe_n[:])
                nc.gpsimd.collective_compute(
                    "AllGather", mybir.AluOpType.bypass, replica_groups=RG,
                    ins=[ag1_in], outs=[ag1_out])
                if DEBUG:
                    nc.sync.dma_start(dbg["dbg_ag1"][:, :], ag1_out[:, :])

        # ================= Stage B: per-head attention =================
        with tc.tile_pool(name="sb", bufs=1) as sb:
            qT = sb.tile([P, 2, S], F32R)
            qpe2 = sb.tile([64, 2, S], F32R)
            kT = sb.tile([P, 2, S], F32R)
            kpeT = sb.tile([64, S], F32R)
            v_tok = sb.tile([P, 2, 16, P], F32R)
            oT = sb.tile([P, 2, S], F32R)
            qpe_raw = sb.tile([P, S], F32)
            vT_raw = sb.tile([P, 2, S], F32R)

            with tc.tile_pool(name="sbw", bufs=1) as sbw, \
                 tc.tile_pool(name="sbr", bufs=3) as sbr, \
                 tc.tile_pool(name="pbs", bufs=2, space="PSUM") as pbs:
                with nc.named_scope("stageB_qkv"):
                    wqb = sbw.tile([P, 12, 3, P], F32R)
                    nc.sync.dma_start(wqb[:], wq_b_t.rearrange("a b p m -> p a b m"))
                    wkb = sbw.tile([P, 4, 4, P], F32R)
                    nc.sync.dma_start(wkb[:], wkv_b_t.rearrange("a b p m -> p a b m"))
                    for blk in range(8):
                        sl = slice(blk * SS, (blk + 1) * SS)
                        rhs_u = sbr.tile([P, 16, SS], F32R, tag="rhs1")
                        nc.sync.dma_start(
                            rhs_u[:],
                            r32(ag1_out[blk * BS:blk * BS + QLR + KVLR].rearrange(
                                "(kt p) s -> p kt s", p=P)))
                        for mc in range(3):
                            ps = pbs.tile([P, SS], F32, tag="qb")
                            for kt in range(12):
                                nc.tensor.matmul(ps[:], wqb[:, kt, mc], rhs_u[:, kt],
                                                 start=(kt == 0), stop=(kt == 11))
                            if mc < 2:
                                nc.vector.tensor_copy(qT[:, mc, sl], ps[:])
                            else:
                                nc.vector.tensor_copy(qpe_raw[:, sl], ps[:])
                        for mc in range(4):
                            ps = pbs.tile([P, SS], F32, tag="qb")
                            for kt in range(4):
                                nc.tensor.matmul(ps[:], wkb[:, kt, mc],
                                                 rhs_u[:, 12 + kt],
                                                 start=(kt == 0), stop=(kt == 3))
                            if mc < 2:
                                nc.vector.tensor_copy(kT[:, mc, sl], ps[:])
                            else:
                                nc.vector.tensor_copy(vT_raw[:, mc - 2, sl], ps[:])
                        nc.sync.dma_start(
                            kpeT[:, sl],
                            r32(ag1_out[blk * BS + QLR + KVLR:blk * BS + BS]))

            with tc.tile_pool(name="sbt", bufs=1) as sbt, \
                 tc.tile_pool(name="pbt", bufs=1, space="PSUM") as pbt:
                with nc.named_scope("stageB_rope"):
                    # rope q_pe (two heads packed in the 128 rows, 32-row quarters)
                    cos_t = sbt.tile([P, S], F32)
                    nc.sync.dma_start(cos_t[:], cossin[0:P, :])
                    sin_t = sbt.tile([P, S], F32)
                    nc.sync.dma_start(sin_t[:], cossin[P:2 * P, :])
                    qsw = sbt.tile([P, S], F32)
                    for qq in range(2):
                        b = qq * 64
                        nc.sync.dma_start(qsw[b:b + 32, :], qpe_raw[b + 32:b + 64, :])
                        nc.sync.dma_start(qsw[b + 32:b + 64, :], qpe_raw[b:b + 32, :])
                    qpe_rot = sbt.tile([P, S], F32R)
                    nc.vector.tensor_mul(qpe_rot[:], qpe_raw[:], cos_t[:, :])
                    t1r = sbt.tile([P, S], F32)
                    nc.vector.tensor_mul(t1r[:], qsw[:], sin_t[:, :])
                    nc.vector.tensor_add(qpe_rot[:], qpe_rot[:], t1r[:])
                    nc.sync.dma_start(qpe2[:, 0], qpe_rot[0:64, :])
                    nc.sync.dma_start(qpe2[:, 1], qpe_rot[64:128, :])

                    # V -> token-major via PE transpose
                    for h in range(2):
                        for st in range(16):
                            pt = pbt.tile([P, P], F32R, tag="vtr", bufs=2)
                            nc.tensor.transpose(
                                pt[:], vT_raw[:, h, st * P:(st + 1) * P], ident_r[:])
                            nc.vector.tensor_copy(v_tok[:, h, st], pt[:])

            with tc.tile_pool(name="sbe", bufs=3) as sbe, \
                 tc.tile_pool(name="sbm", bufs=1) as sbm, \
                 tc.tile_pool(name="pat", bufs=2, space="PSUM") as pat:
                with nc.named_scope("stageB_attn"):
                    mask_t = sbm.tile([P, 4, 512], F32)
                    nc.sync.dma_start(mask_t[:], dmask[:, :, :])
                    for h in range(2):
                        for qc in range(4):
                            qsl = slice(qc * 512, (qc + 1) * 512)
                            o_ps = pat.tile([P, 512], F32, tag="o")
                            d_ps = pat.tile([1, 512], F32, tag="d")
                            nkt = 4 * qc + 4
                            for kt in range(nkt):
                                ksl = slice(kt * P, (kt + 1) * P)
                                sc_ps = pat.tile([P, 512], F32, tag="sc")
                                nc.tensor.matmul(sc_ps[:], kT[:, h, ksl],
                                                 qT[:, h, qsl], start=True, stop=False)
                                nc.tensor.matmul(sc_ps[:], kpeT[:, ksl],
                                                 qpe2[:, h, qsl], start=False, stop=True)
                                j = kt - 4 * qc
                                if j >= 0:
                                    nc.vector.tensor_add(sc_ps[:], sc_ps[:],
                                                         mask_t[:, j])
                                es = sbe.tile([P, 512], F32R, tag="es")
                                nc.scalar.activation(es[:], sc_ps[:], AF.Exp)
                                nc.tensor.matmul(o_ps[:], v_tok[:, h, kt], es[:],
                                                 start=(kt == 0), stop=(kt == nkt - 1))
                                nc.tensor.matmul(d_ps[:], ones_r[:], es[:],
                                                 start=(kt == 0), stop=(kt == nkt - 1))
                            rec = sbe.tile([1, 512], F32R, tag="rec")
                            with nc.allow_low_precision(
                                    reason="f32r rounding of softmax denom"):
                                nc.vector.reciprocal(rec[:], d_ps[:])
                            rb_ps = pat.tile([P, 512], F32, tag="rb")
                            nc.tensor.matmul(rb_ps[:], ones_k1[:], rec[:],
                                             start=True, stop=True)
                            recb = sbe.tile([P, 512], F32, tag="recb")
                            nc.vector.tensor_copy(recb[:], rb_ps[:])
                            nc.vector.tensor_mul(oT[:, h, qsl], o_ps[:], recb[:])
                    nc.sync.dma_start(
                        r32(ag2_in.rearrange("(mc p) s -> p mc s", p=P)), oT[:])
                    nc.gpsimd.collective_compute(
                        "AllGather", mybir.AluOpType.bypass, replica_groups=RG,
                        ins=[ag2_in], outs=[ag2_out])
                    if DEBUG:
                        nc.sync.dma_start(dbg["dbg_ag2"][:, :], ag2_out[:, :])

        # ============ Stage C/D share h2 (residual2 + final add) ============
        with tc.tile_pool(name="h2p", bufs=1) as h2p:
            h2 = h2p.tile([P, 2, S], F32)
            with tc.tile_pool(name="scw", bufs=1) as scw, \
                 tc.tile_pool(name="sc", bufs=1) as sc, \
                 tc.tile_pool(name="scr", bufs=2) as scr, \
                 tc.tile_pool(name="pc", bufs=2, space="PSUM") as pc_:
                with nc.named_scope("stageC"):
                    wos = scw.tile([P, 16, 2, P], F32R)
                    nc.sync.dma_start(wos[:], wo_t.rearrange("a b p m -> p a b m"))
                    resid = sc.tile([P, 2, S], F32)
                    nc.sync.dma_start(resid[:],
                                      hT_r.rearrange("(mc p) s -> p mc s", p=P))
                    for ncol in range(4):
                        nsl = slice(ncol * 512, (ncol + 1) * 512)
                        rhs = scr.tile([P, 16, 512], F32R, tag="rhs2")
                        nc.sync.dma_start(
                            rhs[:], r32(ag2_out[:, nsl].rearrange(
                                "(kt p) s -> p kt s", p=P)))
                        for mc in range(2):
                            ps = pc_.tile([P, 512], F32, tag="omm")
                            for kt in range(16):
                                nc.tensor.matmul(ps[:], wos[:, kt, mc], rhs[:, kt],
                                                 start=(kt == 0), stop=(kt == 15))
                            nc.vector.tensor_add(h2[:, mc, nsl], ps[:],
                                                 resid[:, mc, nsl])

                    sqh = sc.tile([P, 2, S], F32R)
                    nc.vector.tensor_mul(sqh[:], h2[:], h2[:])
                    msq4 = sc.tile([1, S], F32)
                    for ncol in range(4):
                        nsl = slice(ncol * 512, (ncol + 1) * 512)
                        ps4 = pc_.tile([1, 512], F32, tag="m4")
                        for mc in range(2):
                            nc.tensor.matmul(ps4[:], ones_r[:], sqh[:, mc, nsl],
                                             start=(mc == 0), stop=(mc == 1))
                        nc.vector.tensor_copy(msq4[:, nsl], ps4[:])
                    nc.sync.dma_start(ar4_in[:, :], msq4[:])
                    nc.gpsimd.collective_compute(
                        "AllReduce", mybir.AluOpType.add, replica_groups=RG,
                        ins=[ar4_in], outs=[ar4_out])
                    msq4g = sc.tile([1, S], F32)
                    nc.sync.dma_start(msq4g[:], ar4_out[:, :])
                    r4s = sc.tile([1, S], F32)
                    nc.scalar.activation(r4s[:], msq4g[:], AF.Sqrt,
                                         scale=1.0 / H, bias=eps_t[:1])
                    r4 = sc.tile([1, S], F32R)
                    with nc.allow_low_precision(reason="f32r rounding of rms scale"):
                        nc.vector.reciprocal(r4[:], r4s[:])
                    r4b = sc.tile([P, S], F32)
                    for ncol in range(4):
                        nsl = slice(ncol * 512, (ncol + 1) * 512)
                        r4bp = pc_.tile([P, 512], F32, tag="rb")
                        nc.tensor.matmul(r4bp[:], ones_k1[:], r4[:, nsl],
                                         start=True, stop=True)
                        nc.vector.tensor_copy(r4b[:, nsl], r4bp[:])
                    yT = sc.tile([P, 2, S], BF16)
                    nc.vector.tensor_mul(yT[:], h2[:],
                                         r4b[:, None, :].to_broadcast([P, 2, S]))
                    nc.sync.dma_start(
                        ag3_in.rearrange("(mc p) s -> p mc s", p=P), yT[:])
                    nc.gpsimd.collective_compute(
                        "AllGather", mybir.AluOpType.bypass, replica_groups=RG,
                        ins=[ag3_in], outs=[ag3_out])
                    if DEBUG:
                        nc.sync.dma_start(dbg["dbg_ag3"][:, :], ag3_out[:, :])
                        nc.sync.dma_start(
                            dbg["dbg_h2"].rearrange("(mc p) s -> p mc s", p=P),
                            h2[:])

            # ---------------- Stage D: MLP (bf16) ----------------
            with tc.tile_pool(name="sd", bufs=1) as sd:
                act = sd.tile([P, 8, S], BF16)
                with tc.tile_pool(name="sdw", bufs=1) as sdw, \
                     tc.tile_pool(name="sdr", bufs=2) as sdr, \
                     tc.tile_pool(name="sde", bufs=4) as sde, \
                     tc.tile_pool(name="pdg", bufs=2, space="PSUM") as pdg:
                    with nc.named_scope("stageD_gateup"):
                        for half in range(2):
                            wg_s = sdw.tile([P, 16, 4, P], BF16, tag="wgh")
                            wu_s = sdw.tile([P, 16, 4, P], BF16, tag="wuh")
                            for m in range(4):
                                nc.sync.dma_start(
                                    wg_s[:, :, m, :],
                                    wg_t[:, half * 4 + m].rearrange("a p m -> p a m"))
                                nc.sync.dma_start(
                                    wu_s[:, :, m, :],
                                    wu_t[:, half * 4 + m].rearrange("a p m -> p a m"))
                            for ncol in range(4):
                                nsl = slice(ncol * 512, (ncol + 1) * 512)
                                rhs = sdr.tile([P, 16, 512], BF16, tag="rhs3")
                                nc.sync.dma_start(
                                    rhs[:], ag3_out[:, nsl].rearrange(
                                        "(kt p) s -> p kt s", p=P))
                                for m in range(4):
                                    gp = pdg.tile([P, 512], F32, tag=f"g{m % 2}",
                                                  name=f"gps{m}")
                                    up = pdg.tile([P, 512], F32, tag=f"u{m % 2}",
                                                  name=f"ups{m}")
                                    for kt in range(16):
                                        nc.tensor.matmul(
                                            gp[:], wg_s[:, kt, m], rhs[:, kt],
                                            start=(kt == 0), stop=(kt == 15))
                                        nc.tensor.matmul(
                                            up[:], wu_s[:, kt, m], rhs[:, kt],
                                            start=(kt == 0), stop=(kt == 15))
                                    gsil = sde.tile([P, 512], BF16, tag="gsil")
                                    nc.scalar.activation(gsil[:], gp[:], AF.Silu)
                                    nc.vector.tensor_mul(
                                        act[:, half * 4 + m, nsl], gsil[:], up[:])

                with tc.tile_pool(name="sdw2", bufs=1) as sdw2, \
                     tc.tile_pool(name="sdd", bufs=3) as sdd, \
                     tc.tile_pool(name="pdd", bufs=2, space="PSUM") as pdd:
                    with nc.named_scope("stageD_down"):
                        wds = sdw2.tile([P, 8, 16, P], BF16)
                        nc.sync.dma_start(wds[:], wd_t.rearrange("a b p m -> p a b m"))
                        for j in range(4):
                            nsl = slice(j * 512, (j + 1) * 512)
                            for mc in range(16):
                                ps = pdd.tile([P, 512], F32, tag="dmm")
                                for kt in range(8):
                                    nc.tensor.matmul(ps[:], wds[:, kt, mc],
                                                     act[:, kt, nsl],
                                                     start=(kt == 0), stop=(kt == 7))
                                dn = sdd.tile([P, 512], BF16, tag="dn")
                                nc.vector.tensor_copy(dn[:], ps[:])
                                nc.sync.dma_start(
                                    rs_in[j][mc * P:(mc + 1) * P, :], dn[:])
                            nc.gpsimd.collective_compute(
                                "ReduceScatter", mybir.AluOpType.add,
                                replica_groups=RG,
                                ins=[rs_in[j]], outs=[rs_out[j]])
                            fin = sdd.tile([P, 2, 512], BF16, tag="fin")
                            nc.sync.dma_start(
                                fin[:],
                                rs_out[j].rearrange("(mc p) s -> p mc s", p=P))
                            fino = sdd.tile([P, 2, 512], F32, tag="fino")
                            nc.vector.tensor_add(fino[:], fin[:], h2[:, :, nsl])
                            nc.sync.dma_start(
                                outT.rearrange("(mc p) s -> p mc s", p=P)[:, :, nsl],
                                fino[:])

    nc.compile()
    _CACHE["nc"] = nc
    return nc


def _host_prep(inputs):
    import ml_dtypes
    bf16 = ml_dtypes.bfloat16
    inp = {k: np.asarray(v) for k, v in inputs.items()}
    hidden = inp["hidden_states"].reshape(S, H).astype(np.float32)
    pos = inp["position_ids"].reshape(S).astype(np.int64)
    cosT = inp["cos"][pos].T.astype(np.float32)
    sinT = inp["sin"][pos].T.astype(np.float32)
    wq_a = (inp["wq_a"] * inp["in_ln"][:, None]).astype(np.float32)
    wkv_a = (inp["wkv_a"] * inp["in_ln"][:, None]).astype(np.float32)
    wq_b = (inp["wq_b"] * inp["q_a_ln"][:, None]).astype(np.float32)
    wkv_b = (inp["wkv_b"] * inp["kv_a_ln"][:, None]).astype(np.float32)
    wg = (inp["w_gate"] * inp["post_ln"][:, None]).astype(np.float32)
    wu = (inp["w_up"] * inp["post_ln"][:, None]).astype(np.float32)
    wd = inp["w_down"].astype(np.float32)
    wo = inp["wo"].astype(np.float32)

    de = np.empty(ROPE, np.int64)
    de[:32] = np.arange(32) * 2
    de[32:] = np.arange(32) * 2 + 1
    wkv_a = np.concatenate([wkv_a[:, :KVLR], wkv_a[:, KVLR:][:, de]], axis=1)
    wq_b = wq_b.reshape(QLR, NH, QHD)
    wkv_b = wkv_b.reshape(KVLR, NH, NOPE + VHD)

    hT = hidden.T.copy()
    sin_sg = np.concatenate([-sinT[:32], sinT[32:]], axis=0)    # signed for swap trick
    cossin = np.concatenate([cosT, cosT, sin_sg, sin_sg], axis=0)  # (256, S)
    ki = np.arange(P)[:, None]
    qi = np.arange(512)[None, :]
    dmask = np.stack([np.where(qi >= j * P + ki, 0.0, -1e30).astype(np.float32)
                      for j in range(4)], axis=1)               # (128, 4, 512)

    wq_a_t = _tile_w(wq_a)
    wkv_a_t = _tile_w(wkv_a)

    in_maps = []
    for c in range(NC):
        h0, h1 = 2 * c, 2 * c + 1
        qb = np.concatenate([
            wq_b[:, h0, :NOPE], wq_b[:, h1, :NOPE],
            wq_b[:, h0, NOPE:][:, de], wq_b[:, h1, NOPE:][:, de]], axis=1) * SCALE
        kb = np.concatenate([
            wkv_b[:, h0, :NOPE], wkv_b[:, h1, :NOPE],
            wkv_b[:, h0, NOPE:], wkv_b[:, h1, NOPE:]], axis=1)
        ssl = slice(c * SS, (c + 1) * SS)
        cs_sh = np.concatenate([cosT[:, ssl], sin_sg[:, ssl]], axis=0)
        in_maps.append({
            "hT_s": np.ascontiguousarray(hT[:, ssl]),
            "hT_r": np.ascontiguousarray(hT[ssl, :]),
            "wq_a_t": wq_a_t,
            "wkv_a_t": wkv_a_t,
            "wq_b_t": _tile_w(qb.astype(np.float32)),
            "wkv_b_t": _tile_w(kb.astype(np.float32)),
            "wo_t": _tile_w(np.ascontiguousarray(wo[:, ssl])),
            "wg_t": _tile_w(wg[:, c * FFS:(c + 1) * FFS]).astype(bf16),
            "wu_t": _tile_w(wu[:, c * FFS:(c + 1) * FFS]).astype(bf16),
            "wd_t": _tile_w(wd[c * FFS:(c + 1) * FFS, :]).astype(bf16),
            "cossin": cossin,
            "cs_sh": np.ascontiguousarray(cs_sh),
            "dmask": dmask,
        })
    return in_maps


_LAST_RESULT = {}


def kernel(**inputs) -> np.ndarray:
    from concourse.bass_utils import run_bass_kernel_spmd
    nc = _build()
    in_maps = _host_prep(inputs)
    kwargs = {}
    if TRACE:
        import sys, types
        if "antenv.axon_hooks" not in sys.modules:
            try:
                from trn_agent_boot.trn_boot import _ntff_profile_via_ctypes
                mod = types.ModuleType("antenv.axon_hooks")
                _hook = _ntff_profile_via_ctypes('/opt/axon/libaxon_pjrt.so')
                mod.get_axon_ntff_profile_hook = lambda: _hook
                mod.set_axon_ntff_profile_hook = lambda h: None
                sys.modules["antenv.axon_hooks"] = mod
                import antenv
                antenv.axon_hooks = mod
            except Exception:
                pass
        kwargs["trace"] = True
    res = run_bass_kernel_spmd(nc, in_maps, list(range(NC)), **kwargs)
    _LAST_RESULT["res"] = res
    outT = np.concatenate([res.results[c]["outT"] for c in range(NC)], axis=0)
    return np.ascontiguousarray(outT.T)[None].astype(np.float32)


# revision 22
# speedup vs baseline: 1.1728x; 1.1728x over previous
"""DeepseekV3 decoder layer on 8 Trainium2 NeuronCores (Bass/Tile).

Sharding: sequence-parallel low-rank projections (local RMS, one AllGather),
tensor-parallel heads for q_b/kv_b/attention (2 heads/core, transposed-score
layout avoids attention transposes), AllGather of head outputs,
output-feature-sharded o_proj + residual, tiny AllReduce for post-LN stats,
AllGather of the normed MLP input, FF-sharded MLP with chunked ReduceScatter
overlapped with the down-projection.

Attention path matmuls run in float32r (full PE rate at N>=256, ~1.5e-4 rel
err); the MLP runs in bf16 (halves SBUF/DMA; error contribution ~1e-3).
RMS/ln weights and the rope de-interleave are folded into the weight
matrices host-side, so device RMS is exact and rope is 5 vector ops.
"""

import numpy as np

B, S, H = 1, 2048, 2048
NH, NOPE, ROPE, VHD = 16, 128, 64, 128
QHD = NOPE + ROPE
QLR, KVLR, FF = 1536, 512, 8192
SCALE = QHD ** -0.5
EPS = 1e-6
NC = 8
SS = S // NC            # 256: sequence / output-feature shard
FFS = FF // NC          # 1024: FF shard
P = 128
BS = QLR + KVLR + ROPE  # 2112: ag1 block rows per rank

TRACE = False           # test.py sets kernel.TRACE = True for profiling
DEBUG = False           # adds debug ExternalOutputs of stage boundaries

_CACHE = {}


def _tile_w(w):
    """[K, M] -> [K/128, ceil(M/128), 128, 128] contiguous blocks (zero-pad M)."""
    K, M = w.shape
    mc = -(-M // P)
    out = np.zeros((K // P, mc, P, P), np.float32)
    wp = np.zeros((K, mc * P), np.float32)
    wp[:, :M] = w
    for kt in range(K // P):
        for m in range(mc):
            out[kt, m] = wp[kt * P:(kt + 1) * P, m * P:(m + 1) * P]
    return out


def _build():
    if "nc" in _CACHE:
        return _CACHE["nc"]
    import concourse.mybir as mybir
    import concourse.tile as tile
    from concourse import bacc
    from concourse.masks import make_identity

    F32 = mybir.dt.float32
    F32R = mybir.dt.float32r
    BF16 = mybir.dt.bfloat16
    AF = mybir.ActivationFunctionType

    nc = bacc.Bacc("TRN2", target_bir_lowering=False, debug=False, num_devices=NC)

    def inp(name, shape, dt=F32):
        return nc.dram_tensor(name, list(shape), dt, kind="ExternalInput").ap()

    hT_s = inp("hT_s", [H, SS])
    hT_r = inp("hT_r", [SS, S])
    wq_a_t = inp("wq_a_t", [16, 12, P, P], F32R)
    wkv_a_t = inp("wkv_a_t", [16, 5, P, P], F32R)
    wq_b_t = inp("wq_b_t", [12, 3, P, P], BF16)
    wkv_b_t = inp("wkv_b_t", [4, 4, P, P], BF16)
    wo_t = inp("wo_t", [16, 2, P, P], BF16)
    wg_t = inp("wg_t", [16, 8, P, P], BF16)
    wu_t = inp("wu_t", [16, 8, P, P], BF16)
    wd_t = inp("wd_t", [8, 16, P, P], BF16)
    cossin = inp("cossin", [2 * P, S])        # rows 0:128 [cosT;cosT], 128:256 [sinT;sinT]
    cs_sh = inp("cs_sh", [P, SS])             # rows 0:64 cosT, 64:128 sinT (own shard)
    dmask = inp("dmask", [P, 4, 512])
    outT = nc.dram_tensor("outT", [SS, S], F32, kind="ExternalOutput").ap()

    RG = [list(range(NC))]
    dbg = {}
    if DEBUG:
        for nm, shp in [("dbg_ag1", [NC * BS, SS]), ("dbg_ag2", [NH * VHD, S]),
                        ("dbg_ag3", [H, S]), ("dbg_h2", [SS, S])]:
            dt = mybir.dt.bfloat16 if nm == "dbg_ag3" else F32
            dbg[nm] = nc.dram_tensor(nm, shp, dt, kind="ExternalOutput").ap()

    def r32(ap):
        return ap.bitcast(F32R)

    from contextlib import ExitStack
    with tile.TileContext(nc) as tc, ExitStack() as _stack:
        cpool = _stack.enter_context(tc.tile_pool(name="const", bufs=1))
        dpool = _stack.enter_context(tc.tile_pool(name="dram", bufs=1, space="DRAM"))
        ag1_in = dpool.tile([BS, SS], BF16)
        ag1_out = dpool.tile([NC * BS, SS], BF16, addr_space="Shared")
        ag2_in = [dpool.tile([2 * VHD, 512], BF16, name=f"ag2_in{j}")
                  for j in range(4)]
        ag2_out = [dpool.tile([NH * VHD, 512], BF16, addr_space="Shared",
                              name=f"ag2_out{j}") for j in range(4)]
        ar4_in = dpool.tile([1, S], F32)
        ar4_out = dpool.tile([1, S], F32, addr_space="Shared")
        ag3_in = [dpool.tile([SS, 512], BF16, name=f"ag3_in{j}") for j in range(4)]
        ag3_out = [dpool.tile([H, 512], BF16, addr_space="Shared",
                              name=f"ag3_out{j}") for j in range(4)]
        rs_in = [dpool.tile([H, 512], BF16, name=f"rs_in{j}") for j in range(4)]
        rs_out = [dpool.tile([SS, 512], BF16, name=f"rs_out{j}") for j in range(4)]
        ones_f = cpool.tile([P, 1], F32)
        nc.vector.memset(ones_f[:], 1.0)
        ones_r = cpool.tile([P, 1], BF16)
        nc.vector.tensor_copy(ones_r[:], ones_f[:])
        ident_f = cpool.tile([P, P], F32)
        make_identity(nc, ident_f)
        ident_r = cpool.tile([P, P], BF16)
        nc.vector.tensor_copy(ident_r[:], ident_f[:])
        eps_t = cpool.tile([P, 1], F32)
        nc.vector.memset(eps_t[:], EPS)
        ones_k1f = cpool.tile([1, P], F32)
        nc.vector.memset(ones_k1f[:], 1.0)
        ones_k1 = cpool.tile([1, P], F32R)
        nc.vector.tensor_copy(ones_k1[:], ones_k1f[:])

        # ================= Stage A: seq-shard low-rank path =================
        with tc.tile_pool(name="sa", bufs=1) as sa, \
             tc.tile_pool(name="saw", bufs=4) as saw, \
             tc.tile_pool(name="pa", bufs=2, space="PSUM") as pa:
            with nc.named_scope("stageA"):
                xs = sa.tile([P, 16, SS], F32)
                nc.sync.dma_start(xs[:], hT_s.rearrange("(kt p) s -> p kt s", p=P))
                sq = sa.tile([P, 16, SS], BF16)
                nc.vector.tensor_mul(sq[:], xs[:], xs[:])
                msq_ps = pa.tile([1, SS], F32, tag="msq")
                for kt in range(16):
                    nc.tensor.matmul(msq_ps[:], ones_r[:], sq[:, kt],
                                     start=(kt == 0), stop=(kt == 15))
                r1s = sa.tile([1, SS], F32)
                nc.scalar.activation(r1s[:], msq_ps[:], AF.Sqrt, scale=1.0 / H, bias=eps_t[:1])
                r1 = sa.tile([1, SS], F32R)
                with nc.allow_low_precision(reason="f32r rounding of rms scale"):
                    nc.vector.reciprocal(r1[:], r1s[:])
                r1bp = pa.tile([P, SS], F32, tag="rb", name="r1bp")
                nc.tensor.matmul(r1bp[:], ones_k1[:], r1[:], start=True, stop=True)
                r1b = sa.tile([P, SS], F32)
                nc.vector.tensor_copy(r1b[:], r1bp[:])
                xn = sa.tile([P, 16, SS], F32R)
                nc.vector.tensor_mul(xn[:], xs[:],
                                     r1b[:, None, :].to_broadcast([P, 16, SS]))

                us = sa.tile([P, 12, SS], F32)
                for mc in range(12):
                    wt = saw.tile([P, 16, P], F32R, tag="aw")
                    nc.sync.dma_start(wt[:], wq_a_t[:, mc].rearrange("a p m -> p a m"))
                    ps = pa.tile([P, SS], F32, tag="amm")
                    for kt in range(16):
                        nc.tensor.matmul(ps[:], wt[:, kt], xn[:, kt],
                                         start=(kt == 0), stop=(kt == 15))
                    nc.vector.tensor_copy(us[:, mc], ps[:])
                cvs = sa.tile([P, 5, SS], F32)
                for mc in range(5):
                    wt = saw.tile([P, 16, P], F32R, tag="aw")
                    nc.sync.dma_start(wt[:], wkv_a_t[:, mc].rearrange("a p m -> p a m"))
                    ps = pa.tile([P, SS], F32, tag="amm")
                    for kt in range(16):
                        nc.tensor.matmul(ps[:], wt[:, kt], xn[:, kt],
                                         start=(kt == 0), stop=(kt == 15))
                    nc.vector.tensor_copy(cvs[:, mc], ps[:])

                sq2 = sa.tile([P, 12, SS], BF16)
                nc.vector.tensor_mul(sq2[:], us[:], us[:])
                msq2 = pa.tile([1, SS], F32, tag="msq")
                for mc in range(12):
                    nc.tensor.matmul(msq2[:], ones_r[:], sq2[:, mc],
                                     start=(mc == 0), stop=(mc == 11))
                r2s = sa.tile([1, SS], F32)
                nc.scalar.activation(r2s[:], msq2[:], AF.Sqrt, scale=1.0 / QLR, bias=eps_t[:1])
                r2 = sa.tile([1, SS], F32R)
                with nc.allow_low_precision(reason="f32r rounding of rms scale"):
                    nc.vector.reciprocal(r2[:], r2s[:])
                r2bp = pa.tile([P, SS], F32, tag="rb", name="r2bp")
                nc.tensor.matmul(r2bp[:], ones_k1[:], r2[:], start=True, stop=True)
                r2b = sa.tile([P, SS], F32)
                nc.vector.tensor_copy(r2b[:], r2bp[:])
                un = sa.tile([P, 12, SS], BF16)
                nc.vector.tensor_mul(un[:], us[:],
                                     r2b[:, None, :].to_broadcast([P, 12, SS]))

                sq3 = sa.tile([P, 4, SS], BF16)
                nc.vector.tensor_mul(sq3[:], cvs[:, :4], cvs[:, :4])
                msq3 = pa.tile([1, SS], F32, tag="msq")
                for mc in range(4):
                    nc.tensor.matmul(msq3[:], ones_r[:], sq3[:, mc],
                                     start=(mc == 0), stop=(mc == 3))
                r3s = sa.tile([1, SS], F32)
                nc.scalar.activation(r3s[:], msq3[:], AF.Sqrt, scale=1.0 / KVLR, bias=eps_t[:1])
                r3 = sa.tile([1, SS], F32R)
                with nc.allow_low_precision(reason="f32r rounding of rms scale"):
                    nc.vector.reciprocal(r3[:], r3s[:])
                r3bp = pa.tile([P, SS], F32, tag="rb", name="r3bp")
                nc.tensor.matmul(r3bp[:], ones_k1[:], r3[:], start=True, stop=True)
                r3b = sa.tile([P, SS], F32)
                nc.vector.tensor_copy(r3b[:], r3bp[:])
                ckn = sa.tile([P, 4, SS], BF16)
                nc.vector.tensor_mul(ckn[:], cvs[:, :4],
                                     r3b[:, None, :].to_broadcast([P, 4, SS]))

                # k_pe rope on cvs[:64, 4] (cs_sh rows 0:64 cos, 64:128 sin)
                cos_sh = sa.tile([64, SS], F32)
                nc.sync.dma_start(cos_sh[:], cs_sh[0:64, :])
                sin_sh = sa.tile([64, SS], F32)
                nc.sync.dma_start(sin_sh[:], cs_sh[64:128, :])
                ksw = sa.tile([64, SS], F32)
                nc.sync.dma_start(ksw[0:32, :], cvs[32:64, 4])
                nc.sync.dma_start(ksw[32:64, :], cvs[0:32, 4])
                kpe_n = sa.tile([64, SS], BF16)
                nc.vector.tensor_mul(kpe_n[:], cvs[:64, 4], cos_sh[:])
                t1 = sa.tile([64, SS], F32)
                nc.vector.tensor_mul(t1[:], ksw[:], sin_sh[:])
                nc.vector.tensor_add(kpe_n[:], kpe_n[:], t1[:])

                nc.sync.dma_start(
                    ag1_in[0:QLR].rearrange("(mc p) s -> p mc s", p=P), un[:])
                nc.sync.dma_start(
                    ag1_in[QLR:QLR + KVLR].rearrange(
                        "(mc p) s -> p mc s", p=P), ckn[:])
                nc.sync.dma_start(ag1_in[QLR + KVLR:BS], kpe_n[:])
                nc.gpsimd.collective_compute(
                    "AllGather", mybir.AluOpType.bypass, replica_groups=RG,
                    ins=[ag1_in], outs=[ag1_out])
                if DEBUG:
                    nc.sync.dma_start(dbg["dbg_ag1"][:, :], ag1_out[:, :])

        # ================= Stage B: per-head attention =================
        with tc.tile_pool(name="sb", bufs=1) as sb:
            qT = sb.tile([P, 2, S], BF16)
            qpe2 = sb.tile([64, 2, S], BF16)
            kT = sb.tile([P, 2, S], BF16)
            kpeT = sb.tile([64, S], BF16)
            v_tok = sb.tile([P, 2, 16, P], BF16)
            oT = sb.tile([P, 2, S], BF16)
            qpe_raw = sb.tile([P, S], F32)
            vT_raw = sb.tile([P, 2, S], BF16)

            with tc.tile_pool(name="sbw", bufs=1) as sbw, \
                 tc.tile_pool(name="sbr", bufs=3) as sbr, \
                 tc.tile_pool(name="pbs", bufs=2, space="PSUM") as pbs:
                with nc.named_scope("stageB_qkv"):
                    wqb = sbw.tile([P, 12, 3, P], BF16)
                    nc.sync.dma_start(wqb[:], wq_b_t.rearrange("a b p m -> p a b m"))
                    wkb = sbw.tile([P, 4, 4, P], BF16)
                    nc.sync.dma_start(wkb[:], wkv_b_t.rearrange("a b p m -> p a b m"))
                    for blk in range(8):
                        sl = slice(blk * SS, (blk + 1) * SS)
                        rhs_u = sbr.tile([P, 16, SS], BF16, tag="rhs1")
                        nc.sync.dma_start(
                            rhs_u[:],
                            ag1_out[blk * BS:blk * BS + QLR + KVLR].rearrange(
                                "(kt p) s -> p kt s", p=P))
                        for mc in range(3):
                            ps = pbs.tile([P, SS], F32, tag="qb")
                            for kt in range(12):
                                nc.tensor.matmul(ps[:], wqb[:, kt, mc], rhs_u[:, kt],
                                                 start=(kt == 0), stop=(kt == 11))
                            if mc < 2:
                                nc.vector.tensor_copy(qT[:, mc, sl], ps[:])
                            else:
                                nc.vector.tensor_copy(qpe_raw[:, sl], ps[:])
                        for mc in range(4):
                            ps = pbs.tile([P, SS], F32, tag="qb")
                            for kt in range(4):
                                nc.tensor.matmul(ps[:], wkb[:, kt, mc],
                                                 rhs_u[:, 12 + kt],
                                                 start=(kt == 0), stop=(kt == 3))
                            if mc < 2:
                                nc.vector.tensor_copy(kT[:, mc, sl], ps[:])
                            else:
                                nc.vector.tensor_copy(vT_raw[:, mc - 2, sl], ps[:])
                        nc.sync.dma_start(
                            kpeT[:, sl],
                            ag1_out[blk * BS + QLR + KVLR:blk * BS + BS])

            with tc.tile_pool(name="sbt", bufs=1) as sbt, \
                 tc.tile_pool(name="pbt", bufs=1, space="PSUM") as pbt:
                with nc.named_scope("stageB_rope"):
                    # rope q_pe (two heads packed in the 128 rows, 32-row quarters)
                    cos_t = sbt.tile([P, S], F32)
                    nc.sync.dma_start(cos_t[:], cossin[0:P, :])
                    sin_t = sbt.tile([P, S], F32)
                    nc.sync.dma_start(sin_t[:], cossin[P:2 * P, :])
                    qsw = sbt.tile([P, S], F32)
                    for qq in range(2):
                        b = qq * 64
                        nc.sync.dma_start(qsw[b:b + 32, :], qpe_raw[b + 32:b + 64, :])
                        nc.sync.dma_start(qsw[b + 32:b + 64, :], qpe_raw[b:b + 32, :])
                    qpe_rot = sbt.tile([P, S], BF16)
                    nc.vector.tensor_mul(qpe_rot[:], qpe_raw[:], cos_t[:, :])
                    t1r = sbt.tile([P, S], F32)
                    nc.vector.tensor_mul(t1r[:], qsw[:], sin_t[:, :])
                    nc.vector.tensor_add(qpe_rot[:], qpe_rot[:], t1r[:])
                    nc.sync.dma_start(qpe2[:, 0], qpe_rot[0:64, :])
                    nc.sync.dma_start(qpe2[:, 1], qpe_rot[64:128, :])

                    # V -> token-major via PE transpose
                    for h in range(2):
                        for st in range(16):
                            pt = pbt.tile([P, P], BF16, tag="vtr", bufs=2)
                            nc.tensor.transpose(
                                pt[:], vT_raw[:, h, st * P:(st + 1) * P], ident_r[:])
                            nc.vector.tensor_copy(v_tok[:, h, st], pt[:])

            with tc.tile_pool(name="sbe", bufs=3) as sbe, \
                 tc.tile_pool(name="sbm", bufs=1) as sbm, \
                 tc.tile_pool(name="pat", bufs=2, space="PSUM") as pat:
                with nc.named_scope("stageB_attn"):
                    mask_t = sbm.tile([P, 4, 512], F32)
                    nc.sync.dma_start(mask_t[:], dmask[:, :, :])
                    for qc in range(4):
                        qsl = slice(qc * 512, (qc + 1) * 512)
                        for h in range(2):
                            o_ps = pat.tile([P, 512], F32, tag="o")
                            d_ps = pat.tile([1, 512], F32, tag="d")
                            nkt = 4 * qc + 4
                            for kt in range(nkt):
                                ksl = slice(kt * P, (kt + 1) * P)
                                sc_ps = pat.tile([P, 512], F32, tag="sc")
                                nc.tensor.matmul(sc_ps[:], kT[:, h, ksl],
                                                 qT[:, h, qsl], start=True, stop=False)
                                nc.tensor.matmul(sc_ps[:], kpeT[:, ksl],
                                                 qpe2[:, h, qsl], start=False, stop=True)
                                j = kt - 4 * qc
                                if j >= 0:
                                    nc.vector.tensor_add(sc_ps[:], sc_ps[:],
                                                         mask_t[:, j])
                                es = sbe.tile([P, 512], BF16, tag="es")
                                nc.scalar.activation(es[:], sc_ps[:], AF.Exp)
                                nc.tensor.matmul(o_ps[:], v_tok[:, h, kt], es[:],
                                                 start=(kt == 0), stop=(kt == nkt - 1))
                                nc.tensor.matmul(d_ps[:], ones_r[:], es[:],
                                                 start=(kt == 0), stop=(kt == nkt - 1))
                            rec = sbe.tile([1, 512], F32R, tag="rec")
                            with nc.allow_low_precision(
                                    reason="f32r rounding of softmax denom"):
                                nc.vector.reciprocal(rec[:], d_ps[:])
                            rb_ps = pat.tile([P, 512], F32, tag="rb")
                            nc.tensor.matmul(rb_ps[:], ones_k1[:], rec[:],
                                             start=True, stop=True)
                            recb = sbe.tile([P, 512], F32, tag="recb")
                            nc.vector.tensor_copy(recb[:], rb_ps[:])
                            nc.vector.tensor_mul(oT[:, h, qsl], o_ps[:], recb[:])
                        nc.sync.dma_start(
                            ag2_in[qc].rearrange("(mc p) s -> p mc s", p=P),
                            oT[:, :, qsl])
                        nc.gpsimd.collective_compute(
                            "AllGather", mybir.AluOpType.bypass, replica_groups=RG,
                            ins=[ag2_in[qc]], outs=[ag2_out[qc]])

        # ============ Stage C/D share h2 (residual2 + final add) ============
        with tc.tile_pool(name="h2p", bufs=1) as h2p:
            h2 = h2p.tile([P, 2, S], F32)
            with tc.tile_pool(name="scw", bufs=1) as scw, \
                 tc.tile_pool(name="sc", bufs=1) as sc, \
                 tc.tile_pool(name="scr", bufs=2) as scr, \
                 tc.tile_pool(name="pc", bufs=2, space="PSUM") as pc_:
                with nc.named_scope("stageC"):
                    wos = scw.tile([P, 16, 2, P], BF16)
                    nc.sync.dma_start(wos[:], wo_t.rearrange("a b p m -> p a b m"))
                    resid = sc.tile([P, 2, S], F32)
                    nc.sync.dma_start(resid[:],
                                      hT_r.rearrange("(mc p) s -> p mc s", p=P))
                    sqh = sc.tile([P, 2, S], BF16)
                    msq4 = sc.tile([1, S], F32)
                    for ncol in range(4):
                        nsl = slice(ncol * 512, (ncol + 1) * 512)
                        rhs = scr.tile([P, 16, 512], BF16, tag="rhs2")
                        nc.sync.dma_start(
                            rhs[:], ag2_out[ncol].rearrange(
                                "(kt p) s -> p kt s", p=P))
                        for mc in range(2):
                            ps = pc_.tile([P, 512], F32, tag="omm")
                            for kt in range(16):
                                nc.tensor.matmul(ps[:], wos[:, kt, mc], rhs[:, kt],
                                                 start=(kt == 0), stop=(kt == 15))
                            nc.vector.tensor_add(h2[:, mc, nsl], ps[:],
                                                 resid[:, mc, nsl])
                        nc.vector.tensor_mul(sqh[:, :, nsl], h2[:, :, nsl],
                                             h2[:, :, nsl])
                        ps4 = pc_.tile([1, 512], F32, tag="m4")
                        for mc in range(2):
                            nc.tensor.matmul(ps4[:], ones_r[:], sqh[:, mc, nsl],
                                             start=(mc == 0), stop=(mc == 1))
                        nc.vector.tensor_copy(msq4[:, nsl], ps4[:])
                    nc.sync.dma_start(ar4_in[:, :], msq4[:])
                    nc.gpsimd.collective_compute(
                        "AllReduce", mybir.AluOpType.add, replica_groups=RG,
                        ins=[ar4_in], outs=[ar4_out])
                    msq4g = sc.tile([1, S], F32)
                    nc.sync.dma_start(msq4g[:], ar4_out[:, :])
                    r4s = sc.tile([1, S], F32)
                    nc.scalar.activation(r4s[:], msq4g[:], AF.Sqrt,
                                         scale=1.0 / H, bias=eps_t[:1])
                    r4 = sc.tile([1, S], F32R)
                    with nc.allow_low_precision(reason="f32r rounding of rms scale"):
                        nc.vector.reciprocal(r4[:], r4s[:])
                    yT = sc.tile([P, 2, S], BF16)
                    for ncol in range(4):
                        nsl = slice(ncol * 512, (ncol + 1) * 512)
                        r4bp = pc_.tile([P, 512], F32, tag="rb")
                        nc.tensor.matmul(r4bp[:], ones_k1[:], r4[:, nsl],
                                         start=True, stop=True)
                        r4b = sc.tile([P, 512], F32, tag="r4b")
                        nc.vector.tensor_copy(r4b[:], r4bp[:])
                        nc.vector.tensor_mul(
                            yT[:, :, nsl], h2[:, :, nsl],
                            r4b[:, None, :].to_broadcast([P, 2, 512]))
                        nc.sync.dma_start(
                            ag3_in[ncol].rearrange("(mc p) s -> p mc s", p=P),
                            yT[:, :, nsl])
                        nc.gpsimd.collective_compute(
                            "AllGather", mybir.AluOpType.bypass, replica_groups=RG,
                            ins=[ag3_in[ncol]], outs=[ag3_out[ncol]])

            # ---------------- Stage D: MLP (bf16) ----------------
            with tc.tile_pool(name="sd", bufs=1) as sd:
                act = sd.tile([P, 8, S], BF16)
                wds = sd.tile([P, 8, 16, P], BF16)
                nc.sync.dma_start(wds[:], wd_t.rearrange("a b p m -> p a b m"))
                with tc.tile_pool(name="sdw", bufs=1) as sdw, \
                     tc.tile_pool(name="sdr", bufs=2) as sdr, \
                     tc.tile_pool(name="sde", bufs=4) as sde, \
                     tc.tile_pool(name="pdg", bufs=2, space="PSUM") as pdg:
                    with nc.named_scope("stageD_gateup"):
                        for half in range(2):
                            wg_s = sdw.tile([P, 16, 4, P], BF16, tag="wgh")
                            wu_s = sdw.tile([P, 16, 4, P], BF16, tag="wuh")
                            for m in range(4):
                                nc.sync.dma_start(
                                    wg_s[:, :, m, :],
                                    wg_t[:, half * 4 + m].rearrange("a p m -> p a m"))
                                nc.sync.dma_start(
                                    wu_s[:, :, m, :],
                                    wu_t[:, half * 4 + m].rearrange("a p m -> p a m"))
                            for ncol in range(4):
                                nsl = slice(ncol * 512, (ncol + 1) * 512)
                                rhs = sdr.tile([P, 16, 512], BF16, tag="rhs3")
                                nc.sync.dma_start(
                                    rhs[:], ag3_out[ncol].rearrange(
                                        "(kt p) s -> p kt s", p=P))
                                for m in range(4):
                                    gp = pdg.tile([P, 512], F32, tag=f"g{m % 2}",
                                                  name=f"gps{m}")
                                    up = pdg.tile([P, 512], F32, tag=f"u{m % 2}",
                                                  name=f"ups{m}")
                                    for kt in range(16):
                                        nc.tensor.matmul(
                                            gp[:], wg_s[:, kt, m], rhs[:, kt],
                                            start=(kt == 0), stop=(kt == 15))
                                        nc.tensor.matmul(
                                            up[:], wu_s[:, kt, m], rhs[:, kt],
                                            start=(kt == 0), stop=(kt == 15))
                                    gsil = sde.tile([P, 512], BF16, tag="gsil")
                                    nc.scalar.activation(gsil[:], gp[:], AF.Silu)
                                    nc.vector.tensor_mul(
                                        act[:, half * 4 + m, nsl], gsil[:], up[:])

                with tc.tile_pool(name="sdd", bufs=3) as sdd, \
                     tc.tile_pool(name="pdd", bufs=2, space="PSUM") as pdd:
                    with nc.named_scope("stageD_down"):
                        for j in range(4):
                            nsl = slice(j * 512, (j + 1) * 512)
                            for mc in range(16):
                                ps = pdd.tile([P, 512], F32, tag="dmm")
                                for kt in range(8):
                                    nc.tensor.matmul(ps[:], wds[:, kt, mc],
                                                     act[:, kt, nsl],
                                                     start=(kt == 0), stop=(kt == 7))
                                dn = sdd.tile([P, 512], BF16, tag="dn")
                                nc.vector.tensor_copy(dn[:], ps[:])
                                nc.sync.dma_start(
                                    rs_in[j][mc * P:(mc + 1) * P, :], dn[:])
                            nc.gpsimd.collective_compute(
                                "ReduceScatter", mybir.AluOpType.add,
                                replica_groups=RG,
                                ins=[rs_in[j]], outs=[rs_out[j]])
                            fin = sdd.tile([P, 2, 512], BF16, tag="fin")
                            nc.sync.dma_start(
                                fin[:],
                                rs_out[j].rearrange("(mc p) s -> p mc s", p=P))
                            fino = sdd.tile([P, 2, 512], F32, tag="fino")
                            nc.vector.tensor_add(fino[:], fin[:], h2[:, :, nsl])
                            nc.sync.dma_start(
                                outT.rearrange("(mc p) s -> p mc s", p=P)[:, :, nsl],
                                fino[:])

    nc.compile()
    _CACHE["nc"] = nc
    return nc


def _host_prep(inputs):
    import ml_dtypes
    bf16 = ml_dtypes.bfloat16
    inp = {k: np.asarray(v) for k, v in inputs.items()}
    hidden = inp["hidden_states"].reshape(S, H).astype(np.float32)
    pos = inp["position_ids"].reshape(S).astype(np.int64)
    cosT = inp["cos"][pos].T.astype(np.float32)
    sinT = inp["sin"][pos].T.astype(np.float32)
    wq_a = (inp["wq_a"] * inp["in_ln"][:, None]).astype(np.float32)
    wkv_a = (inp["wkv_a"] * inp["in_ln"][:, None]).astype(np.float32)
    wq_b = (inp["wq_b"] * inp["q_a_ln"][:, None]).astype(np.float32)
    wkv_b = (inp["wkv_b"] * inp["kv_a_ln"][:, None]).astype(np.float32)
    wg = (inp["w_gate"] * inp["post_ln"][:, None]).astype(np.float32)
    wu = (inp["w_up"] * inp["post_ln"][:, None]).astype(np.float32)
    wd = inp["w_down"].astype(np.float32)
    wo = inp["wo"].astype(np.float32)

    de = np.empty(ROPE, np.int64)
    de[:32] = np.arange(32) * 2
    de[32:] = np.arange(32) * 2 + 1
    wkv_a = np.concatenate([wkv_a[:, :KVLR], wkv_a[:, KVLR:][:, de]], axis=1)
    wq_b = wq_b.reshape(QLR, NH, QHD)
    wkv_b = wkv_b.reshape(KVLR, NH, NOPE + VHD)

    hT = hidden.T.copy()
    sin_sg = np.concatenate([-sinT[:32], sinT[32:]], axis=0)    # signed for swap trick
    cossin = np.concatenate([cosT, cosT, sin_sg, sin_sg], axis=0)  # (256, S)
    ki = np.arange(P)[:, None]
    qi = np.arange(512)[None, :]
    dmask = np.stack([np.where(qi >= j * P + ki, 0.0, -1e30).astype(np.float32)
                      for j in range(4)], axis=1)               # (128, 4, 512)

    wq_a_t = _tile_w(wq_a)
    wkv_a_t = _tile_w(wkv_a)

    in_maps = []
    for c in range(NC):
        h0, h1 = 2 * c, 2 * c + 1
        qb = np.concatenate([
            wq_b[:, h0, :NOPE], wq_b[:, h1, :NOPE],
            wq_b[:, h0, NOPE:][:, de], wq_b[:, h1, NOPE:][:, de]], axis=1) * SCALE
        kb = np.concatenate([
            wkv_b[:, h0, :NOPE], wkv_b[:, h1, :NOPE],
            wkv_b[:, h0, NOPE:], wkv_b[:, h1, NOPE:]], axis=1)
        ssl = slice(c * SS, (c + 1) * SS)
        cs_sh = np.concatenate([cosT[:, ssl], sin_sg[:, ssl]], axis=0)
        in_maps.append({
            "hT_s": np.ascontiguousarray(hT[:, ssl]),
            "hT_r": np.ascontiguousarray(hT[ssl, :]),
            "wq_a_t": wq_a_t,
            "wkv_a_t": wkv_a_t,
            "wq_b_t": _tile_w(qb.astype(np.float32)).astype(bf16),
            "wkv_b_t": _tile_w(kb.astype(np.float32)).astype(bf16),
            "wo_t": _tile_w(np.ascontiguousarray(wo[:, ssl])).astype(bf16),
            "wg_t": _tile_w(wg[:, c * FFS:(c + 1) * FFS]).astype(bf16),
            "wu_t": _tile_w(wu[:, c * FFS:(c + 1) * FFS]).astype(bf16),
            "wd_t": _tile_w(wd[c * FFS:(c + 1) * FFS, :]).astype(bf16),
            "cossin": cossin,
            "cs_sh": np.ascontiguousarray(cs_sh),
            "dmask": dmask,
        })
    return in_maps


_LAST_RESULT = {}


def kernel(**inputs) -> np.ndarray:
    from concourse.bass_utils import run_bass_kernel_spmd
    nc = _build()
    in_maps = _host_prep(inputs)
    kwargs = {}
    if TRACE:
        import sys, types
        if "antenv.axon_hooks" not in sys.modules:
            try:
                from trn_agent_boot.trn_boot import _ntff_profile_via_ctypes
                mod = types.ModuleType("antenv.axon_hooks")
                _hook = _ntff_profile_via_ctypes('/opt/axon/libaxon_pjrt.so')
                mod.get_axon_ntff_profile_hook = lambda: _hook
                mod.set_axon_ntff_profile_hook = lambda h: None
                sys.modules["antenv.axon_hooks"] = mod
                import antenv
                antenv.axon_hooks = mod
            except Exception:
                pass
        kwargs["trace"] = True
    res = run_bass_kernel_spmd(nc, in_maps, list(range(NC)), **kwargs)
    _LAST_RESULT["res"] = res
    outT = np.concatenate([res.results[c]["outT"] for c in range(NC)], axis=0)
    return np.ascontiguousarray(outT.T)[None].astype(np.float32)
